# revision 44
# baseline (speedup 1.0000x reference)
"""Trainium2 Bass kernel for a 2-layer FC-LSTM (B=512, T=128, D=300, H=1024).

Strategy: model-parallel over the hidden dim (each of 8 cores owns 128
hidden units per layer = 512 gate rows), with the batch processed as two
independent 256-sample halves that are software-pipelined against each
other. Weights and activations are bf16 (fp32 PSUM accumulation and fp32
cell state), so matmuls run at the PE's full 1 cycle/row rate.

Per half and step a single fused AllGather moves [h0(t); h1(t-1)] through
a Shared-output HBM buffer; it is issued ~one full iteration before its
consumers, so collective latency hides under the other half's matmuls.
All data DMAs ride the sync-engine hardware DGE queue (fused 3D access
patterns: one DMA per x-step prefetch, one per gathered h tensor).
"""
import sys

sys.path.insert(0, "/opt/trn_rl_repo")

import os
import numpy as np

import concourse.bass as bass
import concourse.bacc as bacc
import concourse.mybir as mybir
from concourse import tile
from concourse.bass_utils import run_bass_kernel_spmd

B, T, D, H = 512, 128, 300, 1024
NCORES = 8
NH = 2                    # batch halves (independent recurrences)
Bh = B // NH              # 256 batch per half
HL = H // NCORES          # 128 hidden units owned per core (per layer)
GL = 4 * HL               # 512 gate rows owned per core
DK = [128, 128, 44]       # D=300 split into K-chunks
KH = H // 128             # 8 K-chunks over the hidden dim

F32 = mybir.dt.float32
F32R = mybir.dt.float32r
BF16 = mybir.dt.bfloat16
AF = mybir.ActivationFunctionType
ALU = mybir.AluOpType
_NO_COLL = bool(os.environ.get("KERNEL_NO_COLL"))


def _build(t_steps, t_total=None):
    t_total = t_total or t_steps
    nc = bacc.Bacc("TRN2", target_bir_lowering=False, debug=False, num_devices=NCORES)

    # x pre-transposed+padded on host: xTv[t, p, kc, b] = x[b, t, kc*128+p]
    xTv = nc.dram_tensor("xTv", [t_total, 128, 3, B], BF16, kind="ExternalInput")
    w0x = nc.dram_tensor("w0x", [128, 3 * GL], BF16, kind="ExternalInput")
    w0h = nc.dram_tensor("w0h", [128, KH * GL], BF16, kind="ExternalInput")
    w1x = nc.dram_tensor("w1x", [128, KH * GL], BF16, kind="ExternalInput")
    w1h = nc.dram_tensor("w1h", [128, KH * GL], BF16, kind="ExternalInput")
    b0d = nc.dram_tensor("b0d", [HL, 4], F32, kind="ExternalInput")
    b1d = nc.dram_tensor("b1d", [HL, 4], F32, kind="ExternalInput")
    wdec = nc.dram_tensor("wdec", [HL, 1], F32R, kind="ExternalInput")
    out_p = nc.dram_tensor("out_p", [1, B], F32, kind="ExternalOutput")

    rg = [list(range(NCORES))]

    with tile.TileContext(nc) as tc:
        with (
            tc.tile_pool(name="wpool", bufs=1) as wp,
            tc.tile_pool(name="xpool", bufs=2) as xp,
            tc.tile_pool(name="hpool", bufs=2) as hp,
            tc.tile_pool(name="zpool", bufs=2) as zp,
            tc.tile_pool(name="cpool", bufs=2) as cp,
            tc.tile_pool(name="pp", bufs=1, space="PSUM") as pp,
            tc.tile_pool(name="dram", bufs=2, space="DRAM") as dp,
        ):
            w0x_s = wp.tile([128, 3 * GL], BF16, tag="w0x", name="w0x")
            nc.sync.dma_start(w0x_s[:], w0x.ap())
            w0h_s = wp.tile([128, KH * GL], BF16, tag="w0h", name="w0h")
            nc.sync.dma_start(w0h_s[:], w0h.ap())
            w1x_s = wp.tile([128, KH * GL], BF16, tag="w1x", name="w1x")
            nc.sync.dma_start(w1x_s[:], w1x.ap())
            w1h_s = wp.tile([128, KH * GL], BF16, tag="w1h", name="w1h")
            nc.sync.dma_start(w1h_s[:], w1h.ap())
            b0_s = wp.tile([HL, 4], F32, tag="b0", name="b0")
            nc.sync.dma_start(b0_s[:], b0d.ap())
            b1_s = wp.tile([HL, 4], F32, tag="b1", name="b1")
            nc.sync.dma_start(b1_s[:], b1d.ap())
            wdec_s = wp.tile([HL, 1], F32R, tag="wdec", name="wdec")
            nc.sync.dma_start(wdec_s[:], wdec.ap())
            z0 = wp.tile([128, Bh], BF16, tag="z0", name="z0")
            nc.vector.memset(z0[:], 0.0)

            def wx(kc, m):
                return w0x_s[0 : DK[kc], kc * GL + m * 128 : kc * GL + (m + 1) * 128]

            def wh(ws, k, m):
                return ws[0:128, k * GL + m * 128 : k * GL + (m + 1) * 128]

            def psap(ps, m):
                return ps[m // 2][:, (m % 2) * Bh : (m % 2 + 1) * Bh]

            # per-half state
            xts = {}               # (h, t) -> x_t tile [128, 3, Bh]
            h0T = [None] * NH      # gathered h0(t).T [128, KH, Bh]
            h1T = [None] * NH      # gathered h1(t-1).T
            c0p = [None] * NH
            c1p = [None] * NH
            h0b = [None] * NH      # local h0 slice bf16 [128, Bh]
            h1b = [None] * NH
            acc = [None] * NH

            def load_x(h, t):
                xn = xp.tile([128, 3, Bh], BF16, tag=f"xt{h}", name=f"xt{h}")
                nc.sync.dma_start(
                    xn[:, :, :], xTv.ap()[t, :, :, h * Bh : (h + 1) * Bh]
                )
                xts[(h, t)] = xn

            def l0_mm(h, t):
                xtile = xts.pop((h, t))
                ps0 = [
                    pp.tile([128, 2 * Bh], F32, tag=f"ps0{h}{j}", name=f"ps0{h}{j}")
                    for j in range(2)
                ]
                for m in range(4):
                    out = psap(ps0, m)
                    for kc in range(3):
                        nc.tensor.matmul(
                            out,
                            wx(kc, m),
                            xtile[0 : DK[kc], kc, :],
                            start=(kc == 0),
                            stop=(t == 0 and kc == 2),
                        )
                    if t > 0:
                        for k in range(KH):
                            nc.tensor.matmul(
                                out,
                                wh(w0h_s, k, m),
                                h0T[h][:, k, :],
                                start=False,
                                stop=(k == KH - 1),
                            )
                return ps0

            def cell0(h, t, ps0):
                zi = zp.tile([128, Bh], F32, tag=f"zi{h}", name=f"zi{h}")
                zf = zp.tile([128, Bh], F32, tag=f"zf{h}", name=f"zf{h}")
                zg = zp.tile([128, Bh], F32, tag=f"zg{h}", name=f"zg{h}")
                zo = zp.tile([128, Bh], F32, tag=f"zo{h}", name=f"zo{h}")
                nc.scalar.activation(zi[:], psap(ps0, 0), AF.Sigmoid, bias=b0_s[:, 0:1])
                if t > 0:
                    nc.scalar.activation(zf[:], psap(ps0, 1), AF.Sigmoid, bias=b0_s[:, 1:2])
                nc.scalar.activation(zg[:], psap(ps0, 2), AF.Tanh, bias=b0_s[:, 2:3])
                nc.scalar.activation(zo[:], psap(ps0, 3), AF.Sigmoid, bias=b0_s[:, 3:4])
                c0 = cp.tile([128, Bh], F32, tag=f"c0{h}", name=f"c0{h}")
                if t == 0:
                    nc.vector.tensor_mul(c0[:], zi[:], zg[:])
                else:
                    ca = zp.tile([128, Bh], F32, tag=f"ca{h}", name=f"ca{h}")
                    cb = zp.tile([128, Bh], F32, tag=f"cb{h}", name=f"cb{h}")
                    nc.vector.tensor_mul(ca[:], zf[:], c0p[h][:])
                    nc.vector.tensor_mul(cb[:], zi[:], zg[:])
                    nc.vector.tensor_add(c0[:], ca[:], cb[:])
                c0p[h] = c0
                tc0 = zp.tile([128, Bh], F32, tag=f"tc0{h}", name=f"tc0{h}")
                nc.scalar.activation(tc0[:], c0[:], AF.Tanh)
                hb = zp.tile([128, Bh], BF16, tag=f"h0b{h}", name=f"h0b{h}")
                nc.vector.tensor_mul(hb[:], zo[:], tc0[:])
                h0b[h] = hb

            def gather(h, t):
                gi = dp.tile([2, 128, Bh], BF16, tag=f"gIn{h}", name=f"gIn{h}")
                nc.scalar.dma_start(gi[0], h0b[h][:])
                if t > 0:
                    nc.scalar.dma_start(gi[1], h1b[h][:])
                else:
                    nc.scalar.dma_start(gi[1], z0[:])
                go = dp.tile(
                    [KH, 2, 128, Bh], BF16, tag=f"gOut{h}", name=f"gOut{h}",
                    addr_space=("Local" if _NO_COLL else "Shared"),
                )
                if not _NO_COLL:
                    nc.gpsimd.collective_compute(
                        "AllGather", ALU.bypass, replica_groups=rg,
                        ins=[gi.opt()], outs=[go.opt()],
                    )
                else:
                    for c in range(NCORES):
                        nc.gpsimd.dma_start(go[c], gi[:])
                return go

            def loads(h, t, go):
                KS = KH // 2
                hn = hp.tile([128, KH, Bh], BF16, tag=f"h0T{h}", name=f"h0T{h}")
                hm = None
                if t > 0:
                    hm = hp.tile([128, KH, Bh], BF16, tag=f"h1T{h}", name=f"h1T{h}")
                    for s in range(2):
                        nc.sync.dma_start(
                            hm[:, s * KS : (s + 1) * KS, :],
                            go[s * KS : (s + 1) * KS, 1, :, :].transpose([1, 0, 2]),
                        )
                    h1T[h] = hm
                for s in range(2):
                    nc.scalar.dma_start(
                        hn[:, s * KS : (s + 1) * KS, :],
                        go[s * KS : (s + 1) * KS, 0, :, :].transpose([1, 0, 2]),
                    )
                h0T[h] = hn

            def l1_mm(h, t):
                ps1 = [
                    pp.tile([128, 2 * Bh], F32, tag=f"ps1{h}{j}", name=f"ps1{h}{j}")
                    for j in range(2)
                ]
                for m in range(4):
                    out = psap(ps1, m)
                    if t > 0:
                        for k in range(KH):
                            nc.tensor.matmul(
                                out, wh(w1h_s, k, m), h1T[h][:, k, :],
                                start=(k == 0), stop=False,
                            )
                    for k in range(KH):
                        nc.tensor.matmul(
                            out, wh(w1x_s, k, m), h0T[h][:, k, :],
                            start=(t == 0 and k == 0), stop=(k == KH - 1),
                        )
                return ps1

            def cell1(h, t, ps1):
                yi = zp.tile([128, Bh], F32, tag=f"yi{h}", name=f"yi{h}")
                yf = zp.tile([128, Bh], F32, tag=f"yf{h}", name=f"yf{h}")
                yg = zp.tile([128, Bh], F32, tag=f"yg{h}", name=f"yg{h}")
                yo = zp.tile([128, Bh], F32, tag=f"yo{h}", name=f"yo{h}")
                nc.scalar.activation(yi[:], psap(ps1, 0), AF.Sigmoid, bias=b1_s[:, 0:1])
                if t > 0:
                    nc.scalar.activation(yf[:], psap(ps1, 1), AF.Sigmoid, bias=b1_s[:, 1:2])
                nc.scalar.activation(yg[:], psap(ps1, 2), AF.Tanh, bias=b1_s[:, 2:3])
                nc.scalar.activation(yo[:], psap(ps1, 3), AF.Sigmoid, bias=b1_s[:, 3:4])
                c1 = cp.tile([128, Bh], F32, tag=f"c1{h}", name=f"c1{h}")
                if t == 0:
                    nc.vector.tensor_mul(c1[:], yi[:], yg[:])
                else:
                    da = zp.tile([128, Bh], F32, tag=f"da{h}", name=f"da{h}")
                    db = zp.tile([128, Bh], F32, tag=f"db{h}", name=f"db{h}")
                    nc.vector.tensor_mul(da[:], yf[:], c1p[h][:])
                    nc.vector.tensor_mul(db[:], yi[:], yg[:])
                    nc.vector.tensor_add(c1[:], da[:], db[:])
                c1p[h] = c1
                tc1 = zp.tile([128, Bh], F32, tag=f"tc1{h}", name=f"tc1{h}")
                nc.scalar.activation(tc1[:], c1[:], AF.Tanh)
                hf = zp.tile([128, Bh], F32R, tag=f"h1f{h}", name=f"h1f{h}")
                nc.vector.tensor_mul(hf[:], yo[:], tc1[:])
                if t < t_steps - 1:
                    hb = zp.tile([128, Bh], BF16, tag=f"h1b{h}", name=f"h1b{h}")
                    nc.scalar.copy(hb[:], hf[:])
                    h1b[h] = hb
                if t == 0:
                    a = cp.tile([128, Bh], F32R, tag=f"acc{h}", name=f"acc{h}")
                    nc.vector.tensor_copy(a[:], hf[:])
                else:
                    a = cp.tile([128, Bh], F32R, tag=f"acc{h}", name=f"acc{h}")
                    nc.vector.tensor_add(a[:], acc[h][:], hf[:])
                acc[h] = a

            # ---- prologue: step 0 layer0 + first gathers + first loads ----
            for h in range(NH):
                load_x(h, 0)
            gouts = [None] * NH
            for h in range(NH):
                ps0 = l0_mm(h, 0)
                cell0(h, 0, ps0)
                gouts[h] = gather(h, 0)
            for h in range(NH):
                if t_steps > 1:
                    load_x(h, 1)
            for h in range(NH):
                loads(h, 0, gouts[h])

            # ---- main loop: per-half [L1(i); L0(i+1); gather(i+1)]; loads at end ----
            for i in range(t_steps):
                for h in range(NH):
                    if i + 2 < t_steps:
                        load_x(h, i + 2)
                for h in range(NH):
                    if i + 1 < t_steps:
                        ps0n = l0_mm(h, i + 1)
                        cell0(h, i + 1, ps0n)
                    ps1 = l1_mm(h, i)
                    cell1(h, i, ps1)
                    if i + 1 < t_steps:
                        gouts[h] = gather(h, i + 1)
                for h in range(NH):
                    if i + 1 < t_steps:
                        loads(h, i + 1, gouts[h])

            # ---- decoder: out_p = (acc/T) . wdec per half (host sums cores) ----
            psd = pp.tile([128, 2 * Bh], F32, tag="ps000", name="psd")
            for h in range(NH):
                nc.tensor.matmul(
                    psd[0:1, h * Bh : (h + 1) * Bh], wdec_s[:, 0:1], acc[h][:],
                    start=True, stop=True,
                )
            outt = zp.tile([1, B], F32, tag="outt", name="outt")
            nc.scalar.copy(outt[:], psd[0:1, :])
            nc.sync.dma_start(out_p.ap(), outt[:])

    nc.compile()
    return nc


def _prep_inputs(x, W_ih0, W_hh0, b_ih0, b_hh0, W_ih1, W_hh1, b_ih1, b_hh1, W_dec, t_steps, t_total=None):
    import ml_dtypes

    bf16 = ml_dtypes.bfloat16
    t_total = t_total or t_steps
    xT = np.transpose(x[:, :t_total, :], (1, 2, 0)).astype(bf16)  # [T, D, B]
    xTv = np.zeros((t_total, 128, 3, B), bf16)
    for kc in range(3):
        xTv[:, 0 : DK[kc], kc, :] = xT[:, kc * 128 : kc * 128 + DK[kc], :]
    b0 = (b_ih0 + b_hh0).astype(np.float32)
    b1 = (b_ih1 + b_hh1).astype(np.float32)
    in_maps = []
    for c in range(NCORES):
        rows = np.concatenate([g * H + np.arange(c * HL, (c + 1) * HL) for g in range(4)])

        def pack(W, nk):
            Wt = np.ascontiguousarray(W[rows, :].T.astype(np.float32))  # [K_total, GL]
            arr = np.zeros((128, nk * GL), np.float32)
            for k in range(nk):
                kp = min(128, Wt.shape[0] - k * 128)
                arr[0:kp, k * GL : k * GL + GL] = Wt[k * 128 : k * 128 + kp, :]
            return arr.astype(bf16)

        in_maps.append({
            "xTv": xTv,
            "w0x": pack(W_ih0, 3),
            "w0h": pack(W_hh0, KH),
            "w1x": pack(W_ih1, KH),
            "w1h": pack(W_hh1, KH),
            "b0d": np.ascontiguousarray(b0[rows].reshape(4, HL).T),
            "b1d": np.ascontiguousarray(b1[rows].reshape(4, HL).T),
            "wdec": np.ascontiguousarray(
                (W_dec[0, c * HL : (c + 1) * HL] / np.float32(t_steps)).reshape(HL, 1)
            ).astype(np.float32),
        })
    return in_maps


def _run(inputs, t_steps, **spmd_kwargs):
    nc = _build(t_steps)
    in_maps = _prep_inputs(
        inputs["x"], inputs["W_ih0"], inputs["W_hh0"], inputs["b_ih0"], inputs["b_hh0"],
        inputs["W_ih1"], inputs["W_hh1"], inputs["b_ih1"], inputs["b_hh1"], inputs["W_dec"],
        t_steps,
    )
    res = run_bass_kernel_spmd(nc, in_maps, core_ids=list(range(NCORES)), **spmd_kwargs)
    part = sum(res.results[c]["out_p"][0] for c in range(NCORES))
    out = (part + inputs["b_dec"][0]).astype(np.float32).reshape(B, 1)
    return out, res


def kernel(**inputs):
    out, _ = _run(inputs, T)
    return out


# revision 46
# speedup vs baseline: 1.0195x; 1.0195x over previous
"""Trainium2 Bass kernel for a 2-layer FC-LSTM (B=512, T=128, D=300, H=1024).

Strategy: model-parallel over the hidden dim (each of 8 cores owns 128
hidden units per layer = 512 gate rows), with the batch processed as two
independent 256-sample halves that are software-pipelined against each
other. Weights and activations are bf16 (fp32 PSUM accumulation and fp32
cell state), so matmuls run at the PE's full 1 cycle/row rate.

Per half and step a single fused AllGather moves [h0(t); h1(t-1)] through
a Shared-output HBM buffer; it is issued ~one full iteration before its
consumers, so collective latency hides under the other half's matmuls.
All data DMAs ride the sync-engine hardware DGE queue (fused 3D access
patterns: one DMA per x-step prefetch, one per gathered h tensor).
"""
import sys

sys.path.insert(0, "/opt/trn_rl_repo")

import os
import numpy as np

import concourse.bass as bass
import concourse.bacc as bacc
import concourse.mybir as mybir
from concourse import tile
from concourse.bass_utils import run_bass_kernel_spmd

B, T, D, H = 512, 128, 300, 1024
NCORES = 8
NH = 2                    # batch halves (independent recurrences)
Bh = B // NH              # 256 batch per half
HL = H // NCORES          # 128 hidden units owned per core (per layer)
GL = 4 * HL               # 512 gate rows owned per core
DK = [128, 128, 44]       # D=300 split into K-chunks
KH = H // 128             # 8 K-chunks over the hidden dim

F32 = mybir.dt.float32
F32R = mybir.dt.float32r
BF16 = mybir.dt.bfloat16
AF = mybir.ActivationFunctionType
ALU = mybir.AluOpType
_NO_COLL = bool(os.environ.get("KERNEL_NO_COLL"))


def _build(t_steps, t_total=None):
    t_total = t_total or t_steps
    nc = bacc.Bacc("TRN2", target_bir_lowering=False, debug=False, num_devices=NCORES)

    # x pre-transposed+padded on host: xTv[t, p, kc, b] = x[b, t, kc*128+p]
    xTv = nc.dram_tensor("xTv", [t_total, 128, 3, B], BF16, kind="ExternalInput")
    w0x = nc.dram_tensor("w0x", [128, 3 * GL], BF16, kind="ExternalInput")
    w0h = nc.dram_tensor("w0h", [128, KH * GL], BF16, kind="ExternalInput")
    w1x = nc.dram_tensor("w1x", [128, KH * GL], BF16, kind="ExternalInput")
    w1h = nc.dram_tensor("w1h", [128, KH * GL], BF16, kind="ExternalInput")
    b0d = nc.dram_tensor("b0d", [HL, 4], F32, kind="ExternalInput")
    b1d = nc.dram_tensor("b1d", [HL, 4], F32, kind="ExternalInput")
    wdec = nc.dram_tensor("wdec", [HL, 1], F32R, kind="ExternalInput")
    out_p = nc.dram_tensor("out_p", [1, B], F32, kind="ExternalOutput")

    rg = [list(range(NCORES))]

    with tile.TileContext(nc) as tc:
        with (
            tc.tile_pool(name="wpool", bufs=1) as wp,
            tc.tile_pool(name="xpool", bufs=2) as xp,
            tc.tile_pool(name="hpool", bufs=2) as hp,
            tc.tile_pool(name="zpool", bufs=2) as zp,
            tc.tile_pool(name="cpool", bufs=2) as cp,
            tc.tile_pool(name="pp", bufs=1, space="PSUM") as pp,
            tc.tile_pool(name="dram", bufs=2, space="DRAM") as dp,
        ):
            w0x_s = wp.tile([128, 3 * GL], BF16, tag="w0x", name="w0x")
            nc.sync.dma_start(w0x_s[:], w0x.ap())
            w0h_s = wp.tile([128, KH * GL], BF16, tag="w0h", name="w0h")
            nc.sync.dma_start(w0h_s[:], w0h.ap())
            w1x_s = wp.tile([128, KH * GL], BF16, tag="w1x", name="w1x")
            nc.sync.dma_start(w1x_s[:], w1x.ap())
            w1h_s = wp.tile([128, KH * GL], BF16, tag="w1h", name="w1h")
            nc.sync.dma_start(w1h_s[:], w1h.ap())
            b0_s = wp.tile([HL, 4], F32, tag="b0", name="b0")
            nc.sync.dma_start(b0_s[:], b0d.ap())
            b1_s = wp.tile([HL, 4], F32, tag="b1", name="b1")
            nc.sync.dma_start(b1_s[:], b1d.ap())
            wdec_s = wp.tile([HL, 1], F32R, tag="wdec", name="wdec")
            nc.sync.dma_start(wdec_s[:], wdec.ap())
            z0 = wp.tile([128, Bh], BF16, tag="z0", name="z0")
            nc.vector.memset(z0[:], 0.0)

            def wx(kc, m):
                return w0x_s[0 : DK[kc], kc * GL + m * 128 : kc * GL + (m + 1) * 128]

            def wh(ws, k, m):
                return ws[0:128, k * GL + m * 128 : k * GL + (m + 1) * 128]

            def psap(ps, m):
                return ps[m // 2][:, (m % 2) * Bh : (m % 2 + 1) * Bh]

            # per-half state
            xts = {}               # (h, t) -> x_t tile [128, 3, Bh]
            h0T = [None] * NH      # gathered h0(t).T [128, KH, Bh]
            h1T = [None] * NH      # gathered h1(t-1).T
            c0p = [None] * NH
            c1p = [None] * NH
            h0b = [None] * NH      # local h0 slice bf16 [128, Bh]
            h1b = [None] * NH
            acc = [None] * NH

            def load_x(h, t):
                xn = xp.tile([128, 3, Bh], BF16, tag=f"xt{h}", name=f"xt{h}")
                nc.sync.dma_start(
                    xn[:, :, :], xTv.ap()[t, :, :, h * Bh : (h + 1) * Bh]
                )
                xts[(h, t)] = xn

            def l0_mm(h, t):
                xtile = xts.pop((h, t))
                ps0 = [
                    pp.tile([128, 2 * Bh], F32, tag=f"ps0{h}{j}", name=f"ps0{h}{j}")
                    for j in range(2)
                ]
                for m in range(4):
                    out = psap(ps0, m)
                    for kc in range(3):
                        nc.tensor.matmul(
                            out,
                            wx(kc, m),
                            xtile[0 : DK[kc], kc, :],
                            start=(kc == 0),
                            stop=(t == 0 and kc == 2),
                        )
                    if t > 0:
                        for k in range(KH):
                            nc.tensor.matmul(
                                out,
                                wh(w0h_s, k, m),
                                h0T[h][:, k, :],
                                start=False,
                                stop=(k == KH - 1),
                            )
                return ps0

            def cell0(h, t, ps0):
                zi = zp.tile([128, Bh], F32, tag=f"zi{h}", name=f"zi{h}")
                zf = zp.tile([128, Bh], F32, tag=f"zf{h}", name=f"zf{h}")
                zg = zp.tile([128, Bh], F32, tag=f"zg{h}", name=f"zg{h}")
                zo = zp.tile([128, Bh], F32, tag=f"zo{h}", name=f"zo{h}")
                nc.scalar.activation(zi[:], psap(ps0, 0), AF.Sigmoid, bias=b0_s[:, 0:1])
                if t > 0:
                    nc.scalar.activation(zf[:], psap(ps0, 1), AF.Sigmoid, bias=b0_s[:, 1:2])
                nc.scalar.activation(zg[:], psap(ps0, 2), AF.Tanh, bias=b0_s[:, 2:3])
                nc.scalar.activation(zo[:], psap(ps0, 3), AF.Sigmoid, bias=b0_s[:, 3:4])
                c0 = cp.tile([128, Bh], F32, tag=f"c0{h}", name=f"c0{h}")
                if t == 0:
                    nc.vector.tensor_mul(c0[:], zi[:], zg[:])
                else:
                    ca = zp.tile([128, Bh], F32, tag=f"ca{h}", name=f"ca{h}")
                    cb = zp.tile([128, Bh], F32, tag=f"cb{h}", name=f"cb{h}")
                    nc.vector.tensor_mul(ca[:], zf[:], c0p[h][:])
                    nc.vector.tensor_mul(cb[:], zi[:], zg[:])
                    nc.vector.tensor_add(c0[:], ca[:], cb[:])
                c0p[h] = c0
                tc0 = zp.tile([128, Bh], F32, tag=f"tc0{h}", name=f"tc0{h}")
                nc.scalar.activation(tc0[:], c0[:], AF.Tanh)
                hb = zp.tile([128, Bh], BF16, tag=f"h0b{h}", name=f"h0b{h}")
                nc.vector.tensor_mul(hb[:], zo[:], tc0[:])
                h0b[h] = hb

            def gather(h, t):
                gi = dp.tile([2, 128, Bh], BF16, tag=f"gIn{h}", name=f"gIn{h}")
                nc.scalar.dma_start(gi[0], h0b[h][:])
                if t > 0:
                    nc.scalar.dma_start(gi[1], h1b[h][:])
                else:
                    nc.scalar.dma_start(gi[1], z0[:])
                go = dp.tile(
                    [KH, 2, 128, Bh], BF16, tag=f"gOut{h}", name=f"gOut{h}",
                    addr_space=("Local" if _NO_COLL else "Shared"),
                )
                if not _NO_COLL:
                    nc.gpsimd.collective_compute(
                        "AllGather", ALU.bypass, replica_groups=rg,
                        ins=[gi.opt()], outs=[go.opt()],
                    )
                else:
                    for c in range(NCORES):
                        nc.gpsimd.dma_start(go[c], gi[:])
                return go

            def loads(h, t, go):
                KS = KH // 2
                hn = hp.tile([128, KH, Bh], BF16, tag=f"h0T{h}", name=f"h0T{h}")
                hm = None
                if t > 0:
                    hm = hp.tile([128, KH, Bh], BF16, tag=f"h1T{h}", name=f"h1T{h}")
                    for s in range(2):
                        nc.scalar.dma_start(
                            hm[:, s * KS : (s + 1) * KS, :],
                            go[s * KS : (s + 1) * KS, 1, :, :].transpose([1, 0, 2]),
                        )
                    h1T[h] = hm
                for s in range(2):
                    nc.sync.dma_start(
                        hn[:, s * KS : (s + 1) * KS, :],
                        go[s * KS : (s + 1) * KS, 0, :, :].transpose([1, 0, 2]),
                    )
                h0T[h] = hn

            def l1_mm(h, t):
                ps1 = [
                    pp.tile([128, 2 * Bh], F32, tag=f"ps1{h}{j}", name=f"ps1{h}{j}")
                    for j in range(2)
                ]
                for m in range(4):
                    out = psap(ps1, m)
                    if t > 0:
                        for k in range(KH):
                            nc.tensor.matmul(
                                out, wh(w1h_s, k, m), h1T[h][:, k, :],
                                start=(k == 0), stop=False,
                            )
                    for k in range(KH):
                        nc.tensor.matmul(
                            out, wh(w1x_s, k, m), h0T[h][:, k, :],
                            start=(t == 0 and k == 0), stop=(k == KH - 1),
                        )
                return ps1

            def cell1(h, t, ps1):
                yi = zp.tile([128, Bh], F32, tag=f"yi{h}", name=f"yi{h}")
                yf = zp.tile([128, Bh], F32, tag=f"yf{h}", name=f"yf{h}")
                yg = zp.tile([128, Bh], F32, tag=f"yg{h}", name=f"yg{h}")
                yo = zp.tile([128, Bh], F32, tag=f"yo{h}", name=f"yo{h}")
                nc.scalar.activation(yi[:], psap(ps1, 0), AF.Sigmoid, bias=b1_s[:, 0:1])
                if t > 0:
                    nc.scalar.activation(yf[:], psap(ps1, 1), AF.Sigmoid, bias=b1_s[:, 1:2])
                nc.scalar.activation(yg[:], psap(ps1, 2), AF.Tanh, bias=b1_s[:, 2:3])
                nc.scalar.activation(yo[:], psap(ps1, 3), AF.Sigmoid, bias=b1_s[:, 3:4])
                c1 = cp.tile([128, Bh], F32, tag=f"c1{h}", name=f"c1{h}")
                if t == 0:
                    nc.vector.tensor_mul(c1[:], yi[:], yg[:])
                else:
                    da = zp.tile([128, Bh], F32, tag=f"da{h}", name=f"da{h}")
                    db = zp.tile([128, Bh], F32, tag=f"db{h}", name=f"db{h}")
                    nc.vector.tensor_mul(da[:], yf[:], c1p[h][:])
                    nc.vector.tensor_mul(db[:], yi[:], yg[:])
                    nc.vector.tensor_add(c1[:], da[:], db[:])
                c1p[h] = c1
                tc1 = zp.tile([128, Bh], F32, tag=f"tc1{h}", name=f"tc1{h}")
                nc.scalar.activation(tc1[:], c1[:], AF.Tanh)
                hf = zp.tile([128, Bh], F32R, tag=f"h1f{h}", name=f"h1f{h}")
                nc.vector.tensor_mul(hf[:], yo[:], tc1[:])
                if t < t_steps - 1:
                    hb = zp.tile([128, Bh], BF16, tag=f"h1b{h}", name=f"h1b{h}")
                    nc.scalar.copy(hb[:], hf[:])
                    h1b[h] = hb
                if t == 0:
                    a = cp.tile([128, Bh], F32R, tag=f"acc{h}", name=f"acc{h}")
                    nc.vector.tensor_copy(a[:], hf[:])
                else:
                    a = cp.tile([128, Bh], F32R, tag=f"acc{h}", name=f"acc{h}")
                    nc.vector.tensor_add(a[:], acc[h][:], hf[:])
                acc[h] = a

            # ---- prologue: step 0 layer0 + first gathers + first loads ----
            for h in range(NH):
                load_x(h, 0)
            gouts = [None] * NH
            for h in range(NH):
                ps0 = l0_mm(h, 0)
                cell0(h, 0, ps0)
                gouts[h] = gather(h, 0)
            for h in range(NH):
                if t_steps > 1:
                    load_x(h, 1)
            for h in range(NH):
                loads(h, 0, gouts[h])

            # ---- main loop: per-half [L1(i); L0(i+1); gather(i+1)]; loads at end ----
            for i in range(t_steps):
                for h in range(NH):
                    if i + 2 < t_steps:
                        load_x(h, i + 2)
                for h in range(NH):
                    ps1 = l1_mm(h, i)
                    cell1(h, i, ps1)
                    if i + 1 < t_steps:
                        ps0n = l0_mm(h, i + 1)
                        cell0(h, i + 1, ps0n)
                        gouts[h] = gather(h, i + 1)
                for h in range(NH):
                    if i + 1 < t_steps:
                        loads(h, i + 1, gouts[h])

            # ---- decoder: out_p = (acc/T) . wdec per half (host sums cores) ----
            psd = pp.tile([128, 2 * Bh], F32, tag="ps000", name="psd")
            for h in range(NH):
                nc.tensor.matmul(
                    psd[0:1, h * Bh : (h + 1) * Bh], wdec_s[:, 0:1], acc[h][:],
                    start=True, stop=True,
                )
            outt = zp.tile([1, B], F32, tag="outt", name="outt")
            nc.scalar.copy(outt[:], psd[0:1, :])
            nc.sync.dma_start(out_p.ap(), outt[:])

    nc.compile()
    return nc


def _prep_inputs(x, W_ih0, W_hh0, b_ih0, b_hh0, W_ih1, W_hh1, b_ih1, b_hh1, W_dec, t_steps, t_total=None):
    import ml_dtypes

    bf16 = ml_dtypes.bfloat16
    t_total = t_total or t_steps
    xT = np.transpose(x[:, :t_total, :], (1, 2, 0)).astype(bf16)  # [T, D, B]
    xTv = np.zeros((t_total, 128, 3, B), bf16)
    for kc in range(3):
        xTv[:, 0 : DK[kc], kc, :] = xT[:, kc * 128 : kc * 128 + DK[kc], :]
    b0 = (b_ih0 + b_hh0).astype(np.float32)
    b1 = (b_ih1 + b_hh1).astype(np.float32)
    in_maps = []
    for c in range(NCORES):
        rows = np.concatenate([g * H + np.arange(c * HL, (c + 1) * HL) for g in range(4)])

        def pack(W, nk):
            Wt = np.ascontiguousarray(W[rows, :].T.astype(np.float32))  # [K_total, GL]
            arr = np.zeros((128, nk * GL), np.float32)
            for k in range(nk):
                kp = min(128, Wt.shape[0] - k * 128)
                arr[0:kp, k * GL : k * GL + GL] = Wt[k * 128 : k * 128 + kp, :]
            return arr.astype(bf16)

        in_maps.append({
            "xTv": xTv,
            "w0x": pack(W_ih0, 3),
            "w0h": pack(W_hh0, KH),
            "w1x": pack(W_ih1, KH),
            "w1h": pack(W_hh1, KH),
            "b0d": np.ascontiguousarray(b0[rows].reshape(4, HL).T),
            "b1d": np.ascontiguousarray(b1[rows].reshape(4, HL).T),
            "wdec": np.ascontiguousarray(
                (W_dec[0, c * HL : (c + 1) * HL] / np.float32(t_steps)).reshape(HL, 1)
            ).astype(np.float32),
        })
    return in_maps


def _run(inputs, t_steps, **spmd_kwargs):
    nc = _build(t_steps)
    in_maps = _prep_inputs(
        inputs["x"], inputs["W_ih0"], inputs["W_hh0"], inputs["b_ih0"], inputs["b_hh0"],
        inputs["W_ih1"], inputs["W_hh1"], inputs["b_ih1"], inputs["b_hh1"], inputs["W_dec"],
        t_steps,
    )
    res = run_bass_kernel_spmd(nc, in_maps, core_ids=list(range(NCORES)), **spmd_kwargs)
    part = sum(res.results[c]["out_p"][0] for c in range(NCORES))
    out = (part + inputs["b_dec"][0]).astype(np.float32).reshape(B, 1)
    return out, res


def kernel(**inputs):
    out, _ = _run(inputs, T)
    return out


# revision 47
# speedup vs baseline: 1.0250x; 1.0054x over previous
"""Trainium2 Bass kernel for a 2-layer FC-LSTM (B=512, T=128, D=300, H=1024).

Strategy: model-parallel over the hidden dim (each of 8 cores owns 128
hidden units per layer = 512 gate rows), with the batch processed as two
independent 256-sample halves that are software-pipelined against each
other. Weights and activations are bf16 (fp32 PSUM accumulation and fp32
cell state), so matmuls run at the PE's full 1 cycle/row rate.

Per half and step a single fused AllGather moves [h0(t); h1(t-1)] through
a Shared-output HBM buffer; it is issued ~one full iteration before its
consumers, so collective latency hides under the other half's matmuls.
All data DMAs ride the sync-engine hardware DGE queue (fused 3D access
patterns: one DMA per x-step prefetch, one per gathered h tensor).
"""
import sys

sys.path.insert(0, "/opt/trn_rl_repo")

import os
import numpy as np

import concourse.bass as bass
import concourse.bacc as bacc
import concourse.mybir as mybir
from concourse import tile
from concourse.bass_utils import run_bass_kernel_spmd

B, T, D, H = 512, 128, 300, 1024
NCORES = 8
NH = 2                    # batch halves (independent recurrences)
Bh = B // NH              # 256 batch per half
HL = H // NCORES          # 128 hidden units owned per core (per layer)
GL = 4 * HL               # 512 gate rows owned per core
DK = [128, 128, 44]       # D=300 split into K-chunks
KH = H // 128             # 8 K-chunks over the hidden dim

F32 = mybir.dt.float32
F32R = mybir.dt.float32r
BF16 = mybir.dt.bfloat16
AF = mybir.ActivationFunctionType
ALU = mybir.AluOpType
_NO_COLL = bool(os.environ.get("KERNEL_NO_COLL"))


def _build(t_steps, t_total=None):
    t_total = t_total or t_steps
    nc = bacc.Bacc("TRN2", target_bir_lowering=False, debug=False, num_devices=NCORES)

    # x pre-transposed+padded on host: xTv[t, p, kc, b] = x[b, t, kc*128+p]
    xTv = nc.dram_tensor("xTv", [t_total, 128, 3, B], BF16, kind="ExternalInput")
    w0x = nc.dram_tensor("w0x", [128, 3 * GL], BF16, kind="ExternalInput")
    w0h = nc.dram_tensor("w0h", [128, KH * GL], BF16, kind="ExternalInput")
    w1x = nc.dram_tensor("w1x", [128, KH * GL], BF16, kind="ExternalInput")
    w1h = nc.dram_tensor("w1h", [128, KH * GL], BF16, kind="ExternalInput")
    b0d = nc.dram_tensor("b0d", [HL, 4], F32, kind="ExternalInput")
    b1d = nc.dram_tensor("b1d", [HL, 4], F32, kind="ExternalInput")
    wdec = nc.dram_tensor("wdec", [HL, 1], F32R, kind="ExternalInput")
    out_p = nc.dram_tensor("out_p", [1, B], F32, kind="ExternalOutput")

    rg = [list(range(NCORES))]

    with tile.TileContext(nc) as tc:
        with (
            tc.tile_pool(name="wpool", bufs=1) as wp,
            tc.tile_pool(name="xpool", bufs=2) as xp,
            tc.tile_pool(name="hpool", bufs=2) as hp,
            tc.tile_pool(name="zpool", bufs=2) as zp,
            tc.tile_pool(name="cpool", bufs=2) as cp,
            tc.tile_pool(name="pp", bufs=1, space="PSUM") as pp,
            tc.tile_pool(name="dram", bufs=2, space="DRAM") as dp,
        ):
            w0x_s = wp.tile([128, 3 * GL], BF16, tag="w0x", name="w0x")
            nc.sync.dma_start(w0x_s[:], w0x.ap())
            w0h_s = wp.tile([128, KH * GL], BF16, tag="w0h", name="w0h")
            nc.sync.dma_start(w0h_s[:], w0h.ap())
            w1x_s = wp.tile([128, KH * GL], BF16, tag="w1x", name="w1x")
            nc.sync.dma_start(w1x_s[:], w1x.ap())
            w1h_s = wp.tile([128, KH * GL], BF16, tag="w1h", name="w1h")
            nc.sync.dma_start(w1h_s[:], w1h.ap())
            b0_s = wp.tile([HL, 4], F32, tag="b0", name="b0")
            nc.sync.dma_start(b0_s[:], b0d.ap())
            b1_s = wp.tile([HL, 4], F32, tag="b1", name="b1")
            nc.sync.dma_start(b1_s[:], b1d.ap())
            wdec_s = wp.tile([HL, 1], F32R, tag="wdec", name="wdec")
            nc.sync.dma_start(wdec_s[:], wdec.ap())
            z0 = wp.tile([128, Bh], BF16, tag="z0", name="z0")
            nc.vector.memset(z0[:], 0.0)

            def wx(kc, m):
                return w0x_s[0 : DK[kc], kc * GL + m * 128 : kc * GL + (m + 1) * 128]

            def wh(ws, k, m):
                return ws[0:128, k * GL + m * 128 : k * GL + (m + 1) * 128]

            def psap(ps, m):
                return ps[m // 2][:, (m % 2) * Bh : (m % 2 + 1) * Bh]

            # per-half state
            xts = {}               # (h, t) -> x_t tile [128, 3, Bh]
            h0T = [None] * NH      # gathered h0(t).T [128, KH, Bh]
            h1T = [None] * NH      # gathered h1(t-1).T
            c0p = [None] * NH
            c1p = [None] * NH
            h0b = [None] * NH      # local h0 slice bf16 [128, Bh]
            h1b = [None] * NH
            acc = [None] * NH

            def load_x(h, t):
                xn = xp.tile([128, 3, Bh], BF16, tag=f"xt{h}", name=f"xt{h}")
                nc.sync.dma_start(
                    xn[:, :, :], xTv.ap()[t, :, :, h * Bh : (h + 1) * Bh]
                )
                xts[(h, t)] = xn

            def l0_mm(h, t):
                xtile = xts.pop((h, t))
                ps0 = [
                    pp.tile([128, 2 * Bh], F32, tag=f"ps0{h}{j}", name=f"ps0{h}{j}")
                    for j in range(2)
                ]
                for m in range(4):
                    out = psap(ps0, m)
                    for kc in range(3):
                        nc.tensor.matmul(
                            out,
                            wx(kc, m),
                            xtile[0 : DK[kc], kc, :],
                            start=(kc == 0),
                            stop=(t == 0 and kc == 2),
                        )
                    if t > 0:
                        for k in range(KH):
                            nc.tensor.matmul(
                                out,
                                wh(w0h_s, k, m),
                                h0T[h][:, k, :],
                                start=False,
                                stop=(k == KH - 1),
                            )
                return ps0

            def cell0(h, t, ps0):
                zi = zp.tile([128, Bh], F32, tag=f"zi{h}", name=f"zi{h}")
                zf = zp.tile([128, Bh], F32, tag=f"zf{h}", name=f"zf{h}")
                zg = zp.tile([128, Bh], F32, tag=f"zg{h}", name=f"zg{h}")
                zo = zp.tile([128, Bh], F32, tag=f"zo{h}", name=f"zo{h}")
                nc.scalar.activation(zi[:], psap(ps0, 0), AF.Sigmoid, bias=b0_s[:, 0:1])
                if t > 0:
                    nc.scalar.activation(zf[:], psap(ps0, 1), AF.Sigmoid, bias=b0_s[:, 1:2])
                nc.scalar.activation(zg[:], psap(ps0, 2), AF.Tanh, bias=b0_s[:, 2:3])
                nc.scalar.activation(zo[:], psap(ps0, 3), AF.Sigmoid, bias=b0_s[:, 3:4])
                c0 = cp.tile([128, Bh], F32, tag=f"c0{h}", name=f"c0{h}")
                if t == 0:
                    nc.vector.tensor_mul(c0[:], zi[:], zg[:])
                else:
                    ca = zp.tile([128, Bh], F32, tag=f"ca{h}", name=f"ca{h}")
                    cb = zp.tile([128, Bh], F32, tag=f"cb{h}", name=f"cb{h}")
                    nc.vector.tensor_mul(ca[:], zf[:], c0p[h][:])
                    nc.vector.tensor_mul(cb[:], zi[:], zg[:])
                    nc.vector.tensor_add(c0[:], ca[:], cb[:])
                c0p[h] = c0
                tc0 = zp.tile([128, Bh], F32, tag=f"tc0{h}", name=f"tc0{h}")
                nc.scalar.activation(tc0[:], c0[:], AF.Tanh)
                hb = zp.tile([128, Bh], BF16, tag=f"h0b{h}", name=f"h0b{h}")
                nc.vector.tensor_mul(hb[:], zo[:], tc0[:])
                h0b[h] = hb

            def gather(h, t):
                gi = dp.tile([2, 128, Bh], BF16, tag=f"gIn{h}", name=f"gIn{h}")
                nc.scalar.dma_start(gi[0], h0b[h][:])
                if t > 0:
                    nc.scalar.dma_start(gi[1], h1b[h][:])
                else:
                    nc.scalar.dma_start(gi[1], z0[:])
                go = dp.tile(
                    [KH, 2, 128, Bh], BF16, tag=f"gOut{h}", name=f"gOut{h}",
                    addr_space=("Local" if _NO_COLL else "Shared"),
                )
                if not _NO_COLL:
                    nc.gpsimd.collective_compute(
                        "AllGather", ALU.bypass, replica_groups=rg,
                        ins=[gi.opt()], outs=[go.opt()],
                    )
                else:
                    for c in range(NCORES):
                        nc.gpsimd.dma_start(go[c], gi[:])
                return go

            def loads(h, t, go):
                KS = KH // 2
                # half B's gather completes into the next iter; its loads must
                # not block the scalar queue (next iter's activations) — route
                # them through sync, whose next work (x prefetch) tolerates it.
                e1 = nc.scalar if h == 0 else nc.sync
                hn = hp.tile([128, KH, Bh], BF16, tag=f"h0T{h}", name=f"h0T{h}")
                hm = None
                if t > 0:
                    hm = hp.tile([128, KH, Bh], BF16, tag=f"h1T{h}", name=f"h1T{h}")
                    for s in range(2):
                        e1.dma_start(
                            hm[:, s * KS : (s + 1) * KS, :],
                            go[s * KS : (s + 1) * KS, 1, :, :].transpose([1, 0, 2]),
                        )
                    h1T[h] = hm
                for s in range(2):
                    nc.sync.dma_start(
                        hn[:, s * KS : (s + 1) * KS, :],
                        go[s * KS : (s + 1) * KS, 0, :, :].transpose([1, 0, 2]),
                    )
                h0T[h] = hn

            def l1_mm(h, t):
                ps1 = [
                    pp.tile([128, 2 * Bh], F32, tag=f"ps1{h}{j}", name=f"ps1{h}{j}")
                    for j in range(2)
                ]
                for m in range(4):
                    out = psap(ps1, m)
                    if t > 0:
                        for k in range(KH):
                            nc.tensor.matmul(
                                out, wh(w1h_s, k, m), h1T[h][:, k, :],
                                start=(k == 0), stop=False,
                            )
                    for k in range(KH):
                        nc.tensor.matmul(
                            out, wh(w1x_s, k, m), h0T[h][:, k, :],
                            start=(t == 0 and k == 0), stop=(k == KH - 1),
                        )
                return ps1

            def cell1(h, t, ps1):
                yi = zp.tile([128, Bh], F32, tag=f"yi{h}", name=f"yi{h}")
                yf = zp.tile([128, Bh], F32, tag=f"yf{h}", name=f"yf{h}")
                yg = zp.tile([128, Bh], F32, tag=f"yg{h}", name=f"yg{h}")
                yo = zp.tile([128, Bh], F32, tag=f"yo{h}", name=f"yo{h}")
                nc.scalar.activation(yi[:], psap(ps1, 0), AF.Sigmoid, bias=b1_s[:, 0:1])
                if t > 0:
                    nc.scalar.activation(yf[:], psap(ps1, 1), AF.Sigmoid, bias=b1_s[:, 1:2])
                nc.scalar.activation(yg[:], psap(ps1, 2), AF.Tanh, bias=b1_s[:, 2:3])
                nc.scalar.activation(yo[:], psap(ps1, 3), AF.Sigmoid, bias=b1_s[:, 3:4])
                c1 = cp.tile([128, Bh], F32, tag=f"c1{h}", name=f"c1{h}")
                if t == 0:
                    nc.vector.tensor_mul(c1[:], yi[:], yg[:])
                else:
                    da = zp.tile([128, Bh], F32, tag=f"da{h}", name=f"da{h}")
                    db = zp.tile([128, Bh], F32, tag=f"db{h}", name=f"db{h}")
                    nc.vector.tensor_mul(da[:], yf[:], c1p[h][:])
                    nc.vector.tensor_mul(db[:], yi[:], yg[:])
                    nc.vector.tensor_add(c1[:], da[:], db[:])
                c1p[h] = c1
                tc1 = zp.tile([128, Bh], F32, tag=f"tc1{h}", name=f"tc1{h}")
                nc.scalar.activation(tc1[:], c1[:], AF.Tanh)
                hf = zp.tile([128, Bh], F32R, tag=f"h1f{h}", name=f"h1f{h}")
                nc.vector.tensor_mul(hf[:], yo[:], tc1[:])
                if t < t_steps - 1:
                    hb = zp.tile([128, Bh], BF16, tag=f"h1b{h}", name=f"h1b{h}")
                    nc.scalar.copy(hb[:], hf[:])
                    h1b[h] = hb
                if t == 0:
                    a = cp.tile([128, Bh], F32R, tag=f"acc{h}", name=f"acc{h}")
                    nc.vector.tensor_copy(a[:], hf[:])
                else:
                    a = cp.tile([128, Bh], F32R, tag=f"acc{h}", name=f"acc{h}")
                    nc.vector.tensor_add(a[:], acc[h][:], hf[:])
                acc[h] = a

            # ---- prologue: step 0 layer0 + first gathers + first loads ----
            for h in range(NH):
                load_x(h, 0)
            gouts = [None] * NH
            for h in range(NH):
                ps0 = l0_mm(h, 0)
                cell0(h, 0, ps0)
                gouts[h] = gather(h, 0)
            for h in range(NH):
                if t_steps > 1:
                    load_x(h, 1)
            for h in range(NH):
                loads(h, 0, gouts[h])

            # ---- main loop: per-half [L1(i); L0(i+1); gather(i+1)]; loads at end ----
            for i in range(t_steps):
                for h in range(NH):
                    if i + 2 < t_steps:
                        load_x(h, i + 2)
                for h in range(NH):
                    ps1 = l1_mm(h, i)
                    cell1(h, i, ps1)
                    if i + 1 < t_steps:
                        ps0n = l0_mm(h, i + 1)
                        cell0(h, i + 1, ps0n)
                        gouts[h] = gather(h, i + 1)
                for h in range(NH):
                    if i + 1 < t_steps:
                        loads(h, i + 1, gouts[h])

            # ---- decoder: out_p = (acc/T) . wdec per half (host sums cores) ----
            psd = pp.tile([128, 2 * Bh], F32, tag="ps000", name="psd")
            for h in range(NH):
                nc.tensor.matmul(
                    psd[0:1, h * Bh : (h + 1) * Bh], wdec_s[:, 0:1], acc[h][:],
                    start=True, stop=True,
                )
            outt = zp.tile([1, B], F32, tag="outt", name="outt")
            nc.scalar.copy(outt[:], psd[0:1, :])
            nc.sync.dma_start(out_p.ap(), outt[:])

    nc.compile()
    return nc


def _prep_inputs(x, W_ih0, W_hh0, b_ih0, b_hh0, W_ih1, W_hh1, b_ih1, b_hh1, W_dec, t_steps, t_total=None):
    import ml_dtypes

    bf16 = ml_dtypes.bfloat16
    t_total = t_total or t_steps
    xT = np.transpose(x[:, :t_total, :], (1, 2, 0)).astype(bf16)  # [T, D, B]
    xTv = np.zeros((t_total, 128, 3, B), bf16)
    for kc in range(3):
        xTv[:, 0 : DK[kc], kc, :] = xT[:, kc * 128 : kc * 128 + DK[kc], :]
    b0 = (b_ih0 + b_hh0).astype(np.float32)
    b1 = (b_ih1 + b_hh1).astype(np.float32)
    in_maps = []
    for c in range(NCORES):
        rows = np.concatenate([g * H + np.arange(c * HL, (c + 1) * HL) for g in range(4)])

        def pack(W, nk):
            Wt = np.ascontiguousarray(W[rows, :].T.astype(np.float32))  # [K_total, GL]
            arr = np.zeros((128, nk * GL), np.float32)
            for k in range(nk):
                kp = min(128, Wt.shape[0] - k * 128)
                arr[0:kp, k * GL : k * GL + GL] = Wt[k * 128 : k * 128 + kp, :]
            return arr.astype(bf16)

        in_maps.append({
            "xTv": xTv,
            "w0x": pack(W_ih0, 3),
            "w0h": pack(W_hh0, KH),
            "w1x": pack(W_ih1, KH),
            "w1h": pack(W_hh1, KH),
            "b0d": np.ascontiguousarray(b0[rows].reshape(4, HL).T),
            "b1d": np.ascontiguousarray(b1[rows].reshape(4, HL).T),
            "wdec": np.ascontiguousarray(
                (W_dec[0, c * HL : (c + 1) * HL] / np.float32(t_steps)).reshape(HL, 1)
            ).astype(np.float32),
        })
    return in_maps


def _run(inputs, t_steps, **spmd_kwargs):
    nc = _build(t_steps)
    in_maps = _prep_inputs(
        inputs["x"], inputs["W_ih0"], inputs["W_hh0"], inputs["b_ih0"], inputs["b_hh0"],
        inputs["W_ih1"], inputs["W_hh1"], inputs["b_ih1"], inputs["b_hh1"], inputs["W_dec"],
        t_steps,
    )
    res = run_bass_kernel_spmd(nc, in_maps, core_ids=list(range(NCORES)), **spmd_kwargs)
    part = sum(res.results[c]["out_p"][0] for c in range(NCORES))
    out = (part + inputs["b_dec"][0]).astype(np.float32).reshape(B, 1)
    return out, res


def kernel(**inputs):
    out, _ = _run(inputs, T)
    return out
